# revision 32
# baseline (speedup 1.0000x reference)
import numpy as np

B, S, T = 1024, 1024, 32
NCORES = 8
BC = B // NCORES
CHAINS = 2
W = BC // CHAINS
GROUP = 8
CH = 16
RENORM_EVERY = 64
RENORM_LAG = 8
PIECE = 8
SGUARD = 1e-35

_PROG_CACHE = {}
TRACE = False
LAST_RESULTS = None


def _renorm_steps(s_len):
    return set(range(RENORM_EVERY, s_len, RENORM_EVERY))


def _build_program(s_len):
    import concourse.bacc as bacc
    import concourse.mybir as mybir
    from concourse import tile

    f32 = mybir.dt.float32
    bf16 = mybir.dt.bfloat16
    u32 = mybir.dt.uint32
    assert s_len % CH == 0 and s_len % GROUP == 0

    nc = bacc.Bacc("TRN2", target_bir_lowering=False, debug=False,
                   enable_asserts=False, num_devices=NCORES)

    x_dram = nc.dram_tensor("x", [T, s_len * BC], f32, kind="ExternalInput")
    eaug_dram = nc.dram_tensor("eaug", [T, T + 1], bf16, kind="ExternalInput")
    e0t_dram = nc.dram_tensor("e0t", [T, T], bf16, kind="ExternalInput")
    ngroups = s_len // GROUP
    assert ngroups % PIECE == 0
    cap_drams = [nc.dram_tensor(f"cap{ch}", [1, s_len * W], f32,
                                kind="ExternalOutput") for ch in range(CHAINS)]

    renorms = _renorm_steps(s_len)
    nchunks = s_len // CH

    with tile.TileContext(nc) as tc:
        with (
            tc.tile_pool(name="const", bufs=1) as constp,
            tc.tile_pool(name="xs", bufs=3) as xp,
            tc.tile_pool(name="vs", bufs=4) as vp,
            tc.tile_pool(name="caps", bufs=2) as capp,
            tc.tile_pool(name="ps", bufs=3, space="PSUM") as pp,
            tc.tile_pool(name="gs", bufs=1, space="PSUM") as gp,
        ):
            x_tiles = {}

            def ensure_chunk(c):
                if c in x_tiles or c >= nchunks:
                    return
                t = xp.tile([T, CH * BC], f32, tag="xchunk", name="xchunk")
                nc.sync.dma_start(t[:], x_dram[:, c * CH * BC:(c + 1) * CH * BC])
                x_tiles[c] = t

            ensure_chunk(0)
            eaug = constp.tile([T, T + 1], bf16)
            nc.sync.dma_start(eaug[:], eaug_dram[:])
            e0t = constp.tile([T, T], bf16)
            nc.sync.dma_start(e0t[:], e0t_dram[:])
            for c in range(1, min(3, nchunks)):
                ensure_chunk(c)
            rz = [constp.tile([T, W], bf16, name=f"rz{ch}")
                  for ch in range(CHAINS)]
            for ch in range(CHAINS):
                nc.vector.memset(rz[ch][:], 0.0)

            v = []
            for ch in range(CHAINS):
                t = vp.tile([T, W], bf16, tag=f"v{ch}", name=f"v{ch}")
                nc.vector.tensor_copy(t[:], x_tiles[0][:, ch * W:(ch + 1) * W])
                v.append(t)

            cur = [None] * CHAINS
            g_t = [None] * CHAINS
            piece = [None] * CHAINS
            for m in range(1, s_len + 1):
                gi, slot = (m - 1) // GROUP, (m - 1) % GROUP
                if slot == 0:
                    for ch in range(CHAINS):
                        cur[ch] = pp.tile([T + 1, GROUP * W], f32,
                                          tag=f"ps{ch}", name=f"ps{ch}")
                    if gi % PIECE == 0:
                        for ch in range(CHAINS):
                            piece[ch] = capp.tile(
                                [1, PIECE * GROUP * W], f32,
                                tag=f"piece{ch}", name=f"piece{ch}")
                for ch in range(CHAINS):
                    nc.tensor.matmul(cur[ch][:, slot * W:(slot + 1) * W],
                                     eaug[:], v[ch][:])
                if m in renorms:
                    src_slot = (m - RENORM_LAG) % GROUP
                    src_gi = (m - RENORM_LAG) // GROUP
                    assert src_gi == gi, (m, src_gi, gi)
                    for ch in range(CHAINS):
                        tmp = vp.tile([1, W], f32, tag=f"stmp{ch}",
                                      name=f"stmp{ch}")
                        nc.vector.tensor_scalar_max(
                            tmp[:],
                            cur[ch][T:T + 1, src_slot * W:(src_slot + 1) * W],
                            SGUARD)
                        tmp2 = vp.tile([1, W], f32, tag=f"stmp2{ch}",
                                       name=f"stmp2{ch}")
                        nc.vector.tensor_scalar(
                            tmp2[:].bitcast(u32), tmp[:].bitcast(u32),
                            0x7F800000, 0x7F800000,
                            mybir.AluOpType.bitwise_and,
                            mybir.AluOpType.bitwise_xor)
                        nc.vector.tensor_scalar_mul(rz[ch][0:1, :], tmp2[:],
                                                    0.5)
                        g_t[ch] = gp.tile([T, W], f32, tag=f"g{ch}",
                                          name=f"g{ch}")
                        nc.tensor.matmul(g_t[ch][:], e0t[:], rz[ch][:])
                if m < s_len:
                    c = m // CH
                    if m % CH == 0:
                        ensure_chunk(c + 2)
                    for ch in range(CHAINS):
                        xsl = x_tiles[c][:, (m % CH) * BC + ch * W:
                                         (m % CH) * BC + (ch + 1) * W]
                        if m in renorms:
                            t1 = vp.tile([T, W], f32, tag=f"t1{ch}",
                                         name=f"t1{ch}")
                            nc.vector.tensor_mul(
                                t1[:], cur[ch][0:T, slot * W:(slot + 1) * W],
                                xsl)
                            v[ch] = vp.tile([T, W], bf16, tag=f"v{ch}",
                                            name=f"v{ch}")
                            nc.vector.tensor_mul(v[ch][:], t1[:], g_t[ch][:])
                        else:
                            v[ch] = vp.tile([T, W], bf16, tag=f"v{ch}",
                                            name=f"v{ch}")
                            nc.vector.tensor_mul(
                                v[ch][:],
                                cur[ch][0:T, slot * W:(slot + 1) * W], xsl)
                if slot == GROUP - 1:
                    poff = (gi % PIECE) * GROUP * W
                    for ch in range(CHAINS):
                        nc.scalar.copy(piece[ch][:, poff:poff + GROUP * W],
                                       cur[ch][T:T + 1, :])
                    if gi % PIECE == PIECE - 1:
                        base = (gi - (PIECE - 1)) * GROUP * W
                        for ch in range(CHAINS):
                            nc.sync.dma_start(
                                cap_drams[ch][:, base:base + PIECE * GROUP * W],
                                piece[ch][:])

    nc.compile()
    return nc


def _get_program(s_len):
    if s_len not in _PROG_CACHE:
        _PROG_CACHE[s_len] = _build_program(s_len)
    return _PROG_CACHE[s_len]


def _host_prep(em, startt):
    b, s_len, t = em.shape
    x = em.astype(np.float32, copy=True)
    x[:, 0, :] += startt.astype(np.float32)
    mx = x.max(axis=2)
    x -= mx[:, :, None]
    np.exp(x, out=x)
    ssum = x.sum(axis=2)
    x /= ssum[:, :, None]
    a = mx.astype(np.float64) + np.log(ssum.astype(np.float64))
    return x, a


def _device_inputs(x, trans, endt, s_len):
    import ml_dtypes
    bf16 = ml_dtypes.bfloat16
    eaug = np.zeros((T, T + 1), np.float32)
    with np.errstate(under="ignore"):
        eaug[:, :T] = np.exp(trans.astype(np.float64)).astype(np.float32)
        eaug[:, T] = np.exp(endt.astype(np.float64)).astype(np.float32)
    eaug = eaug.astype(bf16)
    e0t = np.zeros((T, T), bf16)
    e0t[0, :] = 1.0
    in_maps = []
    for c in range(NCORES):
        xc = x[c * BC:(c + 1) * BC]
        xt = np.ascontiguousarray(xc.transpose(2, 1, 0)).reshape(T, s_len * BC)
        in_maps.append({"x": xt, "eaug": eaug, "e0t": e0t})
    return in_maps


def _decode_caps(res_core, s_len):
    parts = [res_core[f"cap{ch}"].reshape(s_len, W) for ch in range(CHAINS)]
    return np.concatenate(parts, axis=1)


def _replay_offsets(r_all, s_len):
    o_all = np.zeros((s_len, r_all.shape[1]), np.float64)
    acc = np.zeros(r_all.shape[1], np.float64)
    prev = 0
    for m in sorted(_renorm_steps(s_len)):
        r_used = np.maximum(r_all[m - RENORM_LAG].astype(np.float32),
                            np.float32(SGUARD))
        bits = r_used.view(np.uint32)
        gbits = ((bits & np.uint32(0x7F800000)) ^ np.uint32(0x7F800000))
        ghat = gbits.view(np.float32).astype(np.float64) * 0.5
        o_all[prev:m] = acc
        acc = acc - np.log(ghat)
        prev = m
    o_all[prev:] = acc
    return o_all


def _run_device(x, trans, endt, s_len, trace=False):
    from concourse.bass_utils import run_bass_kernel_spmd

    nc = _get_program(s_len)
    in_maps = _device_inputs(x, trans, endt, s_len)
    res = run_bass_kernel_spmd(nc, in_maps, core_ids=list(range(NCORES)),
                               trace=trace or TRACE)
    global LAST_RESULTS
    LAST_RESULTS = res
    r_all = np.concatenate(
        [_decode_caps(res.results[c], s_len) for c in range(NCORES)],
        axis=1)
    return r_all, res


def _denominator_from_caps(r_all, a, mask, s_len):
    bsz = r_all.shape[1]
    big_a = np.cumsum(a, axis=1)
    o_all = _replay_offsets(r_all, s_len)
    ar = np.arange(s_len)
    tail = np.max(ar[None, :] * mask, axis=1)
    bidx = np.arange(bsz)
    r_tail = np.maximum(r_all[tail, bidx].astype(np.float64), 1e-300)
    den = np.log(r_tail) + big_a[bidx, tail] + o_all[tail, bidx]
    nonempty = mask.sum(axis=1) != 0
    return np.where(nonempty, den, 0.0)


def _numerator(em, tags, mask, startt, trans, endt):
    bsz, s_len, _ = em.shape
    tags = tags.astype(np.int64)
    ar = np.arange(s_len)
    bidx = np.arange(bsz)
    head = np.min(np.where(mask, ar[None, :], s_len - 1), axis=1)
    tail = np.max(ar[None, :] * mask, axis=1)
    nonempty = mask.sum(axis=1) != 0
    cond = mask[:, 1:] & (head[:, None] != ar[None, 1:])
    head_tags = tags[bidx, head]
    tail_tags = tags[bidx, tail]
    em64 = em.astype(np.float64)
    em_tag = np.take_along_axis(em64, tags[:, :, None], axis=2)[:, :, 0]
    trans_step = trans.astype(np.float64)[tags[:, :-1], tags[:, 1:]]
    num = (startt.astype(np.float64)[head_tags]
           + em_tag[bidx, head]
           + np.sum(np.where(cond, trans_step + em_tag[:, 1:], 0.0), axis=1)
           + endt.astype(np.float64)[tail_tags])
    return np.where(nonempty, num, 0.0)


def _finalize(den, num, mask):
    llh = den - num
    labels = mask.sum(axis=1).astype(np.float64)
    eps = 1e-6
    out = np.sum(llh / (labels + eps)) / (np.sum(labels != 0) + eps)
    return np.asarray(out, dtype=np.float32)


def kernel(**inputs):
    em = np.asarray(inputs["emissions"], dtype=np.float32)
    tags = np.asarray(inputs["tags"])
    mask = np.asarray(inputs["mask"]).astype(bool)
    startt = np.asarray(inputs["start_transitions"], dtype=np.float32)
    trans = np.asarray(inputs["transitions"], dtype=np.float32)
    endt = np.asarray(inputs["end_transitions"], dtype=np.float32)
    bsz, s_len, t = em.shape
    assert (bsz, s_len, t) == (B, S, T), (bsz, s_len, t)

    x, a = _host_prep(em, startt)
    r_all, _ = _run_device(x, trans, endt, s_len)
    den = _denominator_from_caps(r_all, a, mask, s_len)
    num = _numerator(em, tags, mask, startt, trans, endt)
    return _finalize(den, num, mask)
